# revision 4
# baseline (speedup 1.0000x reference)
"""Euclidean-distance attention Trainium2 Bass kernel.

Problem: B=8, Sq=Sk=2048, D=512, fp32.
  dist2[q,k]  = ||Q_q||^2 + ||K_k||^2 - 2 Q_q.K_k
  W = softmax(-dist2 / temp, axis=k);  O = W @ V
Outputs: (attended [B,Sq,D], weights [B,Sq,Sk]).

Sharding: data-parallel over batch -> 1 batch element per NeuronCore (8 cores).

Per-core algorithm (softmax is invariant to the per-row constant ||Q_q||^2,
so it is dropped; result is mathematically identical):
  sim'[q,k] = (2 Q.K^T - ||K||^2) / temp
  W = softmax(sim', k);  O = W @ V

Implementation notes:
 - QK^T and W@V run on the PE in float32r (1 cycle/row vs 4 for fp32).
 - The -||K_k||^2/temp term varies along the free (k) dim of S, so it is
   folded into the PSUM accumulation as a K=1 augmented matmul with
   lhsT = ones[1,128].  To keep full precision with float32r operands the
   row is split hi/lo at bf16 granularity (hi exactly representable).
 - Row max via DVE reduce_max (negated), exp+row-sum in one ACT pass
   (accum_out), normalization on DVE.
 - W^T for the PV matmul is built with PE transposes (fp32) of the
   unnormalized exp tile; the 1/l row scale is applied to the PV output.
"""

import numpy as np
from contextlib import ExitStack

import concourse.bass as bass
import concourse.tile as tile
from concourse import bacc, mybir
from concourse.bass import ts
from concourse.bass_utils import run_bass_kernel_spmd
from concourse.masks import make_identity

F32 = mybir.dt.float32
F32R = mybir.dt.float32r
BF16 = mybir.dt.bfloat16
AX = mybir.AxisListType
AF = mybir.ActivationFunctionType

S_FULL = 2048
D_FULL = 512
N_CORES = 8


def build_kernel(S=S_FULL, D=D_FULL, use_f32r=True):
    """Build the single-core program (one batch element)."""
    P = 128
    NT = S // P          # number of 128-row tiles along Sq / Sk
    DC = D // P          # d chunks of 128 (contraction tiles)
    NN = S // 512        # 512-wide n chunks of the S (k) axis
    MMDT = F32R if use_f32r else F32

    nc = bacc.Bacc("TRN2", target_bir_lowering=False, debug=False)
    q_d = nc.dram_tensor("query", [S, D], F32, kind="ExternalInput").ap()
    k_d = nc.dram_tensor("key", [S, D], F32, kind="ExternalInput").ap()
    v_d = nc.dram_tensor("value", [S, D], F32, kind="ExternalInput").ap()
    t_d = nc.dram_tensor("temperature", [1, 1], F32, kind="ExternalInput").ap()
    w_d = nc.dram_tensor("weights", [S, S], F32, kind="ExternalOutput").ap()
    o_d = nc.dram_tensor("attended", [S, D], F32, kind="ExternalOutput").ap()

    with tile.TileContext(nc) as tc, ExitStack() as ctx:
        const = ctx.enter_context(tc.tile_pool(name="const", bufs=1))
        vt_p = ctx.enter_context(tc.tile_pool(name="vt", bufs=1))
        kt_p = ctx.enter_context(tc.tile_pool(name="kt", bufs=1))
        qt_p = ctx.enter_context(tc.tile_pool(name="qt", bufs=1))
        ld_p = ctx.enter_context(tc.tile_pool(name="ld", bufs=3))
        sq_p = ctx.enter_context(tc.tile_pool(name="sq", bufs=2))
        st_p = ctx.enter_context(tc.tile_pool(name="st", bufs=3))
        e_p = ctx.enter_context(tc.tile_pool(name="e", bufs=2))
        w_p = ctx.enter_context(tc.tile_pool(name="w", bufs=2))
        wt_p = ctx.enter_context(tc.tile_pool(name="wtt", bufs=2))
        o_p = ctx.enter_context(tc.tile_pool(name="o", bufs=2))
        dram_p = ctx.enter_context(tc.tile_pool(name="dr", bufs=1, space="DRAM"))
        s_ps = ctx.enter_context(tc.tile_pool(name="sps", bufs=5, space="PSUM"))
        t_ps = ctx.enter_context(tc.tile_pool(name="tps", bufs=2, space="PSUM"))
        o_ps = ctx.enter_context(tc.tile_pool(name="ops", bufs=1, space="PSUM"))

        # ---- constants / scalars ----
        ident = const.tile([P, P], F32, tag="ident")
        make_identity(nc, ident[:])
        ones_f32 = const.tile([1, P], F32, tag="ones_f32")
        nc.gpsimd.memset(ones_f32[:], 1.0)
        ones_col = const.tile([1, P], MMDT, tag="ones_col")
        nc.vector.tensor_copy(ones_col[:], ones_f32[:])

        t_sb = const.tile([1, 1], F32, tag="t_sb")
        inv_t = const.tile([1, 1], F32, tag="inv_t")
        inv_t_b = const.tile([P, 1], F32, tag="inv_t_b")
        two_inv_t = const.tile([P, 1], F32, tag="two_inv_t")
        neg_inv_t = const.tile([P, 1], F32, tag="neg_inv_t")
        nc.sync.dma_start(t_sb[:], t_d[:])
        nc.vector.reciprocal(inv_t[:], t_sb[:])
        nc.gpsimd.partition_broadcast(inv_t_b[:], inv_t[0:1, 0:1])
        nc.scalar.mul(two_inv_t[:], inv_t_b[:], 2.0)
        nc.scalar.mul(neg_inv_t[:], inv_t_b[:], -1.0)

        # ---- V: [S, D] -> SBUF [128, NT*D] (f32r, rounded via copy) ----
        v_all = vt_p.tile([P, NT * D], MMDT, tag="v_all")
        for t in range(NT):
            vn = ld_p.tile([P, D], F32, tag="nat")
            nc.sync.dma_start(vn[:], v_d[ts(t, P), :])
            nc.vector.tensor_copy(v_all[:, ts(t, D)], vn[:])

        # ---- K: load, ||K||^2, transpose into KT (scaled by 2/temp) ----
        # kt_all columns: c*S + k  (chunk c of d on partitions, k along free)
        kt_all = kt_p.tile([P, DC * S], MMDT, tag="kt_all")
        ksq = const.tile([P, NT], F32, tag="ksq")
        for t in range(NT):
            kn = ld_p.tile([P, D], F32, tag="nat")
            nc.sync.dma_start(kn[:], k_d[ts(t, P), :])
            sq = sq_p.tile([P, D], F32, tag="sqs")
            nc.scalar.activation(
                sq[:], kn[:], AF.Square, accum_out=ksq[:, t : t + 1]
            )
            ps = s_ps.tile([P, 512], F32, tag="s")
            for c in range(DC):
                nc.tensor.transpose(ps[:, ts(c, P)], kn[:, ts(c, P)], ident[:])
            # scatter the DC transposed blocks into kt_all at column t*128 of
            # each chunk, scaling by 2/temp on the way out of PSUM
            out_ap = kt_all[:].rearrange("p (c k) -> p c k", k=S)[:, :, ts(t, P)]
            nc.scalar.activation(
                out_ap,
                ps[:].rearrange("p (c j) -> p c j", j=P),
                AF.Copy,
                scale=two_inv_t[:, 0:1],
            )

        # ---- Q: load + transpose into qt_all (q-tile i at cols i*D..) ----
        qt_all = qt_p.tile([P, NT * D], MMDT, tag="qt_all")
        for i in range(NT):
            qn = ld_p.tile([P, D], F32, tag="nat")
            nc.sync.dma_start(qn[:], q_d[ts(i, P), :])
            ps = s_ps.tile([P, 512], F32, tag="s")
            for c in range(DC):
                nc.tensor.transpose(ps[:, ts(c, P)], qn[:, ts(c, P)], ident[:])
            nc.vector.tensor_copy(qt_all[:, ts(i, D)], ps[:])

        # ---- -||K||^2/temp as a [1, S] row, split hi/lo at bf16 ----
        ksq_n = const.tile([P, NT], F32, tag="ksq_n")
        ksq_hb = const.tile([P, NT], BF16, tag="ksq_hb")
        ksq_hi = const.tile([P, NT], F32, tag="ksq_hi")
        ksq_lo = const.tile([P, NT], F32, tag="ksq_lo")
        nc.vector.tensor_scalar_mul(ksq_n[:], ksq[:], neg_inv_t[:, 0:1])
        nc.vector.tensor_copy(ksq_hb[:], ksq_n[:])
        nc.vector.tensor_copy(ksq_hi[:], ksq_hb[:])
        nc.vector.tensor_sub(ksq_lo[:], ksq_n[:], ksq_hi[:])

        ksq_rows = {}
        for name, src in (("hi", ksq_hi), ("lo", ksq_lo)):
            psk = t_ps.tile([NT, P], F32, tag="tps")
            nc.tensor.transpose(psk[:], src[:], ident[:])
            sbk = const.tile([NT, P], MMDT, tag=f"ksqT_{name}")
            nc.vector.tensor_copy(sbk[:], psk[:])
            scr = dram_p.tile([1, S], MMDT, tag=f"scr_{name}")
            nc.sync.dma_start(
                scr[:].rearrange("a (t p) -> (a t) p", p=P), sbk[:]
            )
            row = const.tile([1, S], MMDT, tag=f"ksqrow_{name}")
            nc.sync.dma_start(row[:], scr[:])
            ksq_rows[name] = row

        # ---- main loop over q tiles ----
        for i in range(NT):
            mx = st_p.tile([P, NN], F32, tag="mx")
            l4 = st_p.tile([P, NN], F32, tag="l4")
            negm = st_p.tile([P, 1], F32, tag="negm")
            lsum = st_p.tile([P, 1], F32, tag="lsum")
            linv = st_p.tile([P, 1], F32, tag="linv")

            ps_list = []
            for n in range(NN):
                ps = s_ps.tile([P, 512], F32, tag="s")
                for c in range(DC):
                    nc.tensor.matmul(
                        ps[:],
                        qt_all[:, i * D + c * P : i * D + (c + 1) * P],
                        kt_all[:, c * S + n * 512 : c * S + (n + 1) * 512],
                        start=(c == 0),
                        stop=False,
                    )
                nc.tensor.matmul(
                    ps[:],
                    ones_col[:],
                    ksq_rows["hi"][:, ts(n, 512)],
                    start=False,
                    stop=False,
                )
                nc.tensor.matmul(
                    ps[:],
                    ones_col[:],
                    ksq_rows["lo"][:, ts(n, 512)],
                    start=False,
                    stop=True,
                )
                nc.vector.reduce_max(mx[:, n : n + 1], ps[:], axis=AX.X)
                ps_list.append(ps)

            nc.vector.reduce_max(negm[:], mx[:], axis=AX.X, negate=True)

            e_t = e_p.tile([P, S], F32, tag="e")
            for n in range(NN):
                nc.scalar.activation(
                    e_t[:, ts(n, 512)],
                    ps_list[n][:],
                    AF.Exp,
                    bias=negm[:, 0:1],
                    accum_out=l4[:, n : n + 1],
                )
            nc.vector.reduce_sum(lsum[:], l4[:], axis=AX.X)
            nc.vector.reciprocal(linv[:], lsum[:])

            # normalized weights -> DRAM
            w_t = w_p.tile([P, S], F32, tag="w")
            nc.vector.tensor_scalar_mul(w_t[:], e_t[:], linv[:, 0:1])
            nc.sync.dma_start(w_d[ts(i, P), :], w_t[:])

            # W^T tiles (transpose unnormalized exp)
            wt_t = wt_p.tile([P, S], MMDT, tag="wt")
            for b in range(S // 512):
                wps = t_ps.tile([P, 512], F32, tag="tps")
                for j2 in range(4):
                    j = b * 4 + j2
                    nc.tensor.transpose(
                        wps[:, ts(j2, P)], e_t[:, ts(j, P)], ident[:]
                    )
                nc.scalar.copy(wt_t[:, ts(b, 512)], wps[:])

            # O = (e^T)^T @ V, then scale rows by 1/l
            opsum = o_ps.tile([P, D], F32, tag="o_ps")
            for j in range(NT):
                nc.tensor.matmul(
                    opsum[:],
                    wt_t[:, ts(j, P)],
                    v_all[:, ts(j, D)],
                    start=(j == 0),
                    stop=(j == NT - 1),
                )
            o_t = o_p.tile([P, D], F32, tag="o")
            nc.vector.tensor_scalar_mul(o_t[:], opsum[:], linv[:, 0:1])
            nc.sync.dma_start(o_d[ts(i, P), :], o_t[:])

    nc.compile()
    return nc


_NC_CACHE = {}


def get_nc(S=S_FULL, D=D_FULL, use_f32r=True):
    key = (S, D, use_f32r)
    if key not in _NC_CACHE:
        _NC_CACHE[key] = build_kernel(S, D, use_f32r)
    return _NC_CACHE[key]


def kernel(query, key, value, temperature):
    query = np.asarray(query, dtype=np.float32)
    key = np.asarray(key, dtype=np.float32)
    value = np.asarray(value, dtype=np.float32)
    t = np.asarray(temperature, dtype=np.float32).reshape(1, 1)
    B, S, D = query.shape

    nc = get_nc(S, D)
    in_maps = [
        {
            "query": np.ascontiguousarray(query[b]),
            "key": np.ascontiguousarray(key[b]),
            "value": np.ascontiguousarray(value[b]),
            "temperature": t,
        }
        for b in range(B)
    ]
    res = run_bass_kernel_spmd(nc, in_maps, core_ids=list(range(B)))
    attended = np.stack([res.results[b]["attended"] for b in range(B)])
    weights = np.stack([res.results[b]["weights"] for b in range(B)])
    return attended, weights


# revision 8
# speedup vs baseline: 146.7408x; 146.7408x over previous
"""Euclidean-distance attention Trainium2 Bass kernel.

Problem: B=8, Sq=Sk=2048, D=512, fp32.
  dist2[q,k]  = ||Q_q||^2 + ||K_k||^2 - 2 Q_q.K_k
  W = softmax(-dist2 / temp, axis=k);  O = W @ V
Outputs: (attended [B,Sq,D], weights [B,Sq,Sk]).

Sharding: data-parallel over batch -> 1 batch element per NeuronCore (8 cores).

Per-core algorithm (softmax is invariant to the per-row constant ||Q_q||^2,
so it is dropped; mathematically identical result):
  sim[q,k] = (2 Q.K^T - ||K||^2) / temp
  W = softmax(sim, k);  O = W @ V

Engine mapping per 128-row q tile:
  PE : QK^T in float32r (1 cy/row), W^T transposes in bf16 (1 cy/row),
       W@V in bf16 (1 cy/row)
  DVE: (S_psum - ksq_bcast) subtract, row max, PV output row scale
  ACT: exp with per-partition bias = -rowmax and fused row-sum (accum_out),
       half of the PSUM->SBUF W^T copies, prolog transpose copies
  GPS: ksq broadcast, weight normalization (1/l row scale)
"""

import numpy as np
from contextlib import ExitStack

import concourse.bass as bass
import concourse.tile as tile
from concourse import bacc, mybir
from concourse.bass import ts
from concourse.bass_utils import run_bass_kernel_spmd
from concourse.masks import make_identity

F32 = mybir.dt.float32
F32R = mybir.dt.float32r
BF16 = mybir.dt.bfloat16
AX = mybir.AxisListType
AF = mybir.ActivationFunctionType

S_FULL = 2048
D_FULL = 512
N_CORES = 8


def build_kernel(S=S_FULL, D=D_FULL, use_f32r=True, repeat=1):
    """Build the single-core program (one batch element)."""
    P = 128
    NT = S // P          # 128-row tiles along Sq / Sk
    DC = D // P          # 128-deep contraction chunks of d
    NN = S // 512        # 512-wide chunks of the k axis
    MMDT = F32R if use_f32r else F32

    nc = bacc.Bacc("TRN2", target_bir_lowering=False, debug=False)
    q_d = nc.dram_tensor("query", [S, D], F32, kind="ExternalInput").ap()
    k_d = nc.dram_tensor("key", [S, D], F32, kind="ExternalInput").ap()
    v_d = nc.dram_tensor("value", [S, D], F32, kind="ExternalInput").ap()
    t_d = nc.dram_tensor("temperature", [1, 1], F32, kind="ExternalInput").ap()
    w_d = nc.dram_tensor("weights", [S, S], F32, kind="ExternalOutput").ap()
    o_d = nc.dram_tensor("attended", [S, D], F32, kind="ExternalOutput").ap()

    with tile.TileContext(nc) as tc, ExitStack() as ctx:
        const = ctx.enter_context(tc.tile_pool(name="const", bufs=1))
        vt_p = ctx.enter_context(tc.tile_pool(name="vt", bufs=1))
        kt_p = ctx.enter_context(tc.tile_pool(name="kt", bufs=1))
        qt_p = ctx.enter_context(tc.tile_pool(name="qt", bufs=1))
        ld_p = ctx.enter_context(tc.tile_pool(name="ld", bufs=3))
        sq_p = ctx.enter_context(tc.tile_pool(name="sq", bufs=2))
        st_p = ctx.enter_context(tc.tile_pool(name="st", bufs=3))
        sim_p = ctx.enter_context(tc.tile_pool(name="sim", bufs=2))
        e_p = ctx.enter_context(tc.tile_pool(name="e", bufs=2))
        w_p = ctx.enter_context(tc.tile_pool(name="w", bufs=2))
        wt_p = ctx.enter_context(tc.tile_pool(name="wtt", bufs=2))
        o_p = ctx.enter_context(tc.tile_pool(name="o", bufs=2))
        dram_p = ctx.enter_context(tc.tile_pool(name="dr", bufs=1, space="DRAM"))
        s_ps = ctx.enter_context(tc.tile_pool(name="sps", bufs=5, space="PSUM"))
        t_ps = ctx.enter_context(tc.tile_pool(name="tps", bufs=2, space="PSUM"))
        o_ps = ctx.enter_context(tc.tile_pool(name="ops", bufs=1, space="PSUM"))

        import contextlib
        loop_cm = tc.For_i(0, repeat, 1) if repeat > 1 else contextlib.nullcontext()
        with loop_cm:
            # ---- constants / scalars ----
            ident = const.tile([P, P], F32, tag="ident")
            make_identity(nc, ident[:])
            ident_bf = const.tile([P, P], BF16, tag="ident_bf")
            nc.vector.tensor_copy(ident_bf[:], ident[:])

            t_sb = const.tile([1, 1], F32, tag="t_sb")
            inv_t = const.tile([1, 1], F32, tag="inv_t")
            inv_t_b = const.tile([P, 1], F32, tag="inv_t_b")
            two_inv_t = const.tile([P, 1], F32, tag="two_inv_t")
            nc.sync.dma_start(t_sb[:], t_d[:])
            nc.vector.reciprocal(inv_t[:], t_sb[:])
            nc.gpsimd.partition_broadcast(inv_t_b[:], inv_t[0:1, 0:1])
            nc.scalar.mul(two_inv_t[:], inv_t_b[:], 2.0)

            # ---- V: [S, D] -> SBUF [128, NT*D] bf16 (PV matmul rhs) ----
            v_all = vt_p.tile([P, NT * D], BF16, tag="v_all")
            for t in range(NT):
                vn = ld_p.tile([P, D], F32, tag="nat")
                nc.sync.dma_start(vn[:], v_d[ts(t, P), :])
                nc.scalar.copy(v_all[:, ts(t, D)], vn[:])

            # ---- K: load, ||K||^2, transpose into KT (scaled by 2/temp) ----
            # kt_all columns: c*S + k  (d chunk c on partitions, k along free)
            kt_all = kt_p.tile([P, DC * S], MMDT, tag="kt_all")
            ksq = const.tile([P, NT], F32, tag="ksq")
            for t in range(NT):
                kn = ld_p.tile([P, D], F32, tag="nat")
                nc.sync.dma_start(kn[:], k_d[ts(t, P), :])
                sq = sq_p.tile([P, D], F32, tag="sqs")
                nc.scalar.activation(
                    sq[:], kn[:], AF.Square, accum_out=ksq[:, t : t + 1]
                )
                ps = s_ps.tile([P, 512], F32, tag="s")
                for c in range(DC):
                    nc.tensor.transpose(ps[:, ts(c, P)], kn[:, ts(c, P)], ident[:])
                out_ap = kt_all[:].rearrange("p (c k) -> p c k", k=S)[:, :, ts(t, P)]
                nc.scalar.activation(
                    out_ap,
                    ps[:].rearrange("p (c j) -> p c j", j=P),
                    AF.Copy,
                    scale=two_inv_t[:, 0:1],
                )

            # ---- Q: load + transpose into qt_all (q tile i at cols i*D..) ----
            qt_all = qt_p.tile([P, NT * D], MMDT, tag="qt_all")
            for i in range(NT):
                qn = ld_p.tile([P, D], F32, tag="nat")
                nc.sync.dma_start(qn[:], q_d[ts(i, P), :])
                ps = s_ps.tile([P, 512], F32, tag="s")
                for c in range(DC):
                    nc.tensor.transpose(ps[:, ts(c, P)], qn[:, ts(c, P)], ident[:])
                nc.scalar.copy(qt_all[:, ts(i, D)], ps[:])

            # ---- ||K||^2/temp broadcast to [128, S] (full fp32) ----
            ksq_n = const.tile([P, NT], F32, tag="ksq_n")
            nc.vector.tensor_scalar_mul(ksq_n[:], ksq[:], inv_t_b[:, 0:1])
            psk = t_ps.tile([NT, P], F32, tag="tps")
            nc.tensor.transpose(psk[:], ksq_n[:], ident[:])
            sbk = const.tile([NT, P], F32, tag="ksqT")
            nc.vector.tensor_copy(sbk[:], psk[:])
            scr = dram_p.tile([1, S], F32, tag="scr")
            nc.sync.dma_start(scr[:].rearrange("a (t p) -> (a t) p", p=P), sbk[:])
            ksq_row = const.tile([1, S], F32, tag="ksq_row")
            nc.sync.dma_start(ksq_row[:], scr[:])
            ksq_bc = const.tile([P, S], F32, tag="ksq_bc")
            nc.gpsimd.partition_broadcast(ksq_bc[:], ksq_row[:])

            # ---- main loop over q tiles ----
            for i in range(NT):
                mx = st_p.tile([P, NN], F32, tag="mx")
                l4 = st_p.tile([P, NN], F32, tag="l4")
                negm = st_p.tile([P, 1], F32, tag="negm")
                lsum = st_p.tile([P, 1], F32, tag="lsum")
                linv = st_p.tile([P, 1], F32, tag="linv")

                sim_t = sim_p.tile([P, S], F32, tag="sim")
                for n in range(NN):
                    ps = s_ps.tile([P, 512], F32, tag="s")
                    for c in range(DC):
                        nc.tensor.matmul(
                            ps[:],
                            qt_all[:, i * D + c * P : i * D + (c + 1) * P],
                            kt_all[:, c * S + n * 512 : c * S + (n + 1) * 512],
                            start=(c == 0),
                            stop=(c == DC - 1),
                        )
                    # sim = S - ksq/temp (frees PSUM), then row max
                    nc.vector.tensor_sub(
                        sim_t[:, ts(n, 512)], ps[:], ksq_bc[:, ts(n, 512)]
                    )
                    nc.vector.reduce_max(
                        mx[:, n : n + 1], sim_t[:, ts(n, 512)], axis=AX.X
                    )

                nc.vector.reduce_max(negm[:], mx[:], axis=AX.X, negate=True)

                e_t = e_p.tile([P, S], BF16, tag="e")
                for n in range(NN):
                    nc.scalar.activation(
                        e_t[:, ts(n, 512)],
                        sim_t[:, ts(n, 512)],
                        AF.Exp,
                        bias=negm[:, 0:1],
                        accum_out=l4[:, n : n + 1],
                    )
                nc.vector.reduce_sum(lsum[:], l4[:], axis=AX.X)
                nc.vector.reciprocal(linv[:], lsum[:])

                # normalized fp32 weights -> DRAM (scale on gpsimd)
                w_t = w_p.tile([P, S], F32, tag="w")
                nc.gpsimd.tensor_scalar_mul(w_t[:], e_t[:], linv[:, 0:1])
                nc.sync.dma_start(w_d[ts(i, P), :], w_t[:])

                # W^T tiles in bf16 (transpose unnormalized exp)
                wt_t = wt_p.tile([P, S], BF16, tag="wt")
                for b in range(S // 512):
                    wps = t_ps.tile([P, 512], BF16, tag="tps")
                    for j2 in range(4):
                        j = b * 4 + j2
                        nc.tensor.transpose(
                            wps[:, ts(j2, P)], e_t[:, ts(j, P)], ident_bf[:]
                        )
                    if b % 2 == 0:
                        nc.scalar.copy(wt_t[:, ts(b, 512)], wps[:])
                    else:
                        nc.vector.tensor_copy(wt_t[:, ts(b, 512)], wps[:])

                # O = (e^T)^T @ V then row-scale by 1/l
                opsum = o_ps.tile([P, D], F32, tag="o_ps")
                for j in range(NT):
                    nc.tensor.matmul(
                        opsum[:],
                        wt_t[:, ts(j, P)],
                        v_all[:, ts(j, D)],
                        start=(j == 0),
                        stop=(j == NT - 1),
                    )
                o_t = o_p.tile([P, D], F32, tag="o")
                nc.vector.tensor_scalar_mul(o_t[:], opsum[:], linv[:, 0:1])
                nc.sync.dma_start(o_d[ts(i, P), :], o_t[:])

    nc.compile()
    return nc


_NC_CACHE = {}


def get_nc(S=S_FULL, D=D_FULL, use_f32r=True):
    key = (S, D, use_f32r)
    if key not in _NC_CACHE:
        _NC_CACHE[key] = build_kernel(S, D, use_f32r)
    return _NC_CACHE[key]


def kernel(query, key, value, temperature):
    query = np.asarray(query, dtype=np.float32)
    key = np.asarray(key, dtype=np.float32)
    value = np.asarray(value, dtype=np.float32)
    t = np.asarray(temperature, dtype=np.float32).reshape(1, 1)
    B, S, D = query.shape

    nc = get_nc(S, D)
    in_maps = [
        {
            "query": np.ascontiguousarray(query[b]),
            "key": np.ascontiguousarray(key[b]),
            "value": np.ascontiguousarray(value[b]),
            "temperature": t,
        }
        for b in range(B)
    ]
    res = run_bass_kernel_spmd(nc, in_maps, core_ids=list(range(B)))
    attended = np.stack([res.results[b]["attended"] for b in range(B)])
    weights = np.stack([res.results[b]["weights"] for b in range(B)])
    return attended, weights


# revision 9
# speedup vs baseline: 294.2190x; 2.0050x over previous
"""Euclidean-distance attention Trainium2 Bass kernel.

Problem: B=8, Sq=Sk=2048, D=512, fp32.
  dist2[q,k]  = ||Q_q||^2 + ||K_k||^2 - 2 Q_q.K_k
  W = softmax(-dist2 / temp, axis=k);  O = W @ V
Outputs: (attended [B,Sq,D], weights [B,Sq,Sk]).

Sharding: data-parallel over batch -> 1 batch element per NeuronCore (8 cores).

Per-core algorithm (softmax is invariant to the per-row constant ||Q_q||^2,
so it is dropped; mathematically identical result):
  sim[q,k] = (2 Q.K^T - ||K||^2) / temp
  W = softmax(sim, k);  O = W @ V

Engine mapping per 128-row q tile:
  PE : QK^T in float32r (1 cy/row), W^T transposes in bf16 (1 cy/row),
       W@V in bf16 (1 cy/row)
  DVE: (S_psum - ksq_bcast) subtract, row max, PV output row scale
  ACT: exp with per-partition bias = -rowmax and fused row-sum (accum_out),
       half of the PSUM->SBUF W^T copies, prolog transpose copies
  GPS: ksq broadcast, weight normalization (1/l row scale)
"""

import numpy as np
from contextlib import ExitStack

import concourse.bass as bass
import concourse.tile as tile
from concourse import bacc, mybir
from concourse.bass import ts
from concourse.bass_utils import run_bass_kernel_spmd
from concourse.masks import make_identity

F32 = mybir.dt.float32
F32R = mybir.dt.float32r
BF16 = mybir.dt.bfloat16
AX = mybir.AxisListType
AF = mybir.ActivationFunctionType

S_FULL = 2048
D_FULL = 512
N_CORES = 8


def build_kernel(S=S_FULL, D=D_FULL, use_f32r=True, repeat=1):
    """Build the single-core program (one batch element)."""
    P = 128
    NT = S // P          # 128-row tiles along Sq / Sk
    DC = D // P          # 128-deep contraction chunks of d
    NN = S // 512        # 512-wide chunks of the k axis
    MMDT = F32R if use_f32r else F32

    nc = bacc.Bacc("TRN2", target_bir_lowering=False, debug=False)
    q_d = nc.dram_tensor("query", [S, D], F32, kind="ExternalInput").ap()
    k_d = nc.dram_tensor("key", [S, D], F32, kind="ExternalInput").ap()
    v_d = nc.dram_tensor("value", [S, D], F32, kind="ExternalInput").ap()
    t_d = nc.dram_tensor("temperature", [1, 1], F32, kind="ExternalInput").ap()
    w_d = nc.dram_tensor("weights", [S, S], F32, kind="ExternalOutput").ap()
    o_d = nc.dram_tensor("attended", [S, D], F32, kind="ExternalOutput").ap()

    with tile.TileContext(nc) as tc, ExitStack() as ctx:
        const = ctx.enter_context(tc.tile_pool(name="const", bufs=1))
        vt_p = ctx.enter_context(tc.tile_pool(name="vt", bufs=1))
        kt_p = ctx.enter_context(tc.tile_pool(name="kt", bufs=1))
        qt_p = ctx.enter_context(tc.tile_pool(name="qt", bufs=1))
        ld_p = ctx.enter_context(tc.tile_pool(name="ld", bufs=3))
        sq_p = ctx.enter_context(tc.tile_pool(name="sq", bufs=2))
        st_p = ctx.enter_context(tc.tile_pool(name="st", bufs=4))
        sim_p = ctx.enter_context(tc.tile_pool(name="sim", bufs=3))
        e_p = ctx.enter_context(tc.tile_pool(name="e", bufs=3))
        w_p = ctx.enter_context(tc.tile_pool(name="w", bufs=2))
        wt_p = ctx.enter_context(tc.tile_pool(name="wtt", bufs=3))
        o_p = ctx.enter_context(tc.tile_pool(name="o", bufs=2))
        dram_p = ctx.enter_context(tc.tile_pool(name="dr", bufs=1, space="DRAM"))
        s_ps = ctx.enter_context(tc.tile_pool(name="sps", bufs=5, space="PSUM"))
        t_ps = ctx.enter_context(tc.tile_pool(name="tps", bufs=2, space="PSUM"))
        o_ps = ctx.enter_context(tc.tile_pool(name="ops", bufs=1, space="PSUM"))

        import contextlib
        loop_cm = tc.For_i(0, repeat, 1) if repeat > 1 else contextlib.nullcontext()
        with loop_cm:
            # ---- constants / scalars ----
            ident = const.tile([P, P], F32, tag="ident")
            make_identity(nc, ident[:])
            ident_bf = const.tile([P, P], BF16, tag="ident_bf")
            nc.vector.tensor_copy(ident_bf[:], ident[:])

            t_sb = const.tile([1, 1], F32, tag="t_sb")
            inv_t = const.tile([1, 1], F32, tag="inv_t")
            inv_t_b = const.tile([P, 1], F32, tag="inv_t_b")
            two_inv_t = const.tile([P, 1], F32, tag="two_inv_t")
            nc.sync.dma_start(t_sb[:], t_d[:])
            nc.vector.reciprocal(inv_t[:], t_sb[:])
            nc.gpsimd.partition_broadcast(inv_t_b[:], inv_t[0:1, 0:1])
            nc.scalar.mul(two_inv_t[:], inv_t_b[:], 2.0)

            # ---- V: [S, D] -> SBUF [128, NT*D] bf16 (PV matmul rhs) ----
            v_all = vt_p.tile([P, NT * D], BF16, tag="v_all")
            for t in range(NT):
                vn = ld_p.tile([P, D], F32, tag="nat")
                nc.sync.dma_start(vn[:], v_d[ts(t, P), :])
                nc.scalar.copy(v_all[:, ts(t, D)], vn[:])

            # ---- K: load, ||K||^2, transpose into KT (scaled by 2/temp) ----
            # kt_all columns: c*S + k  (d chunk c on partitions, k along free)
            kt_all = kt_p.tile([P, DC * S], MMDT, tag="kt_all")
            ksq = const.tile([P, NT], F32, tag="ksq")
            for t in range(NT):
                kn = ld_p.tile([P, D], F32, tag="nat")
                nc.sync.dma_start(kn[:], k_d[ts(t, P), :])
                sq = sq_p.tile([P, D], F32, tag="sqs")
                nc.scalar.activation(
                    sq[:], kn[:], AF.Square, accum_out=ksq[:, t : t + 1]
                )
                ps = s_ps.tile([P, 512], F32, tag="s")
                for c in range(DC):
                    nc.tensor.transpose(ps[:, ts(c, P)], kn[:, ts(c, P)], ident[:])
                out_ap = kt_all[:].rearrange("p (c k) -> p c k", k=S)[:, :, ts(t, P)]
                nc.scalar.activation(
                    out_ap,
                    ps[:].rearrange("p (c j) -> p c j", j=P),
                    AF.Copy,
                    scale=two_inv_t[:, 0:1],
                )

            # ---- Q: load + transpose into qt_all (q tile i at cols i*D..) ----
            qt_all = qt_p.tile([P, NT * D], MMDT, tag="qt_all")
            for i in range(NT):
                qn = ld_p.tile([P, D], F32, tag="nat")
                nc.sync.dma_start(qn[:], q_d[ts(i, P), :])
                ps = s_ps.tile([P, 512], F32, tag="s")
                for c in range(DC):
                    nc.tensor.transpose(ps[:, ts(c, P)], qn[:, ts(c, P)], ident[:])
                nc.scalar.copy(qt_all[:, ts(i, D)], ps[:])

            # ---- ||K||^2/temp broadcast to [128, S] (full fp32) ----
            ksq_n = const.tile([P, NT], F32, tag="ksq_n")
            nc.vector.tensor_scalar_mul(ksq_n[:], ksq[:], inv_t_b[:, 0:1])
            psk = t_ps.tile([NT, P], F32, tag="tps")
            nc.tensor.transpose(psk[:], ksq_n[:], ident[:])
            sbk = const.tile([NT, P], F32, tag="ksqT")
            nc.vector.tensor_copy(sbk[:], psk[:])
            scr = dram_p.tile([1, S], F32, tag="scr")
            nc.sync.dma_start(scr[:].rearrange("a (t p) -> (a t) p", p=P), sbk[:])
            ksq_row = const.tile([1, S], F32, tag="ksq_row")
            nc.sync.dma_start(ksq_row[:], scr[:])
            ksq_bc = const.tile([P, S], F32, tag="ksq_bc")
            nc.gpsimd.partition_broadcast(ksq_bc[:], ksq_row[:])

            # ---- main loop over q tiles ----
            for i in range(NT):
                mx = st_p.tile([P, NN], F32, tag="mx")
                l4 = st_p.tile([P, NN], F32, tag="l4")
                negm = st_p.tile([P, 1], F32, tag="negm")
                lsum = st_p.tile([P, 1], F32, tag="lsum")
                linv = st_p.tile([P, 1], F32, tag="linv")

                sim_t = sim_p.tile([P, S], F32, tag="sim")
                for n in range(NN):
                    ps = s_ps.tile([P, 512], F32, tag="s")
                    for c in range(DC):
                        nc.tensor.matmul(
                            ps[:],
                            qt_all[:, i * D + c * P : i * D + (c + 1) * P],
                            kt_all[:, c * S + n * 512 : c * S + (n + 1) * 512],
                            start=(c == 0),
                            stop=(c == DC - 1),
                        )
                    # sim = S - ksq/temp (frees PSUM), then row max
                    nc.vector.tensor_sub(
                        sim_t[:, ts(n, 512)], ps[:], ksq_bc[:, ts(n, 512)]
                    )
                    nc.vector.reduce_max(
                        mx[:, n : n + 1], sim_t[:, ts(n, 512)], axis=AX.X
                    )

                nc.vector.reduce_max(negm[:], mx[:], axis=AX.X, negate=True)

                e_t = e_p.tile([P, S], BF16, tag="e")
                for n in range(NN):
                    nc.scalar.activation(
                        e_t[:, ts(n, 512)],
                        sim_t[:, ts(n, 512)],
                        AF.Exp,
                        bias=negm[:, 0:1],
                        accum_out=l4[:, n : n + 1],
                    )
                nc.vector.reduce_sum(lsum[:], l4[:], axis=AX.X)
                nc.vector.reciprocal(linv[:], lsum[:])

                # W^T tiles in bf16 (transpose unnormalized exp) -- emitted
                # first so the PE-feeding path gets scheduler priority
                wt_t = wt_p.tile([P, S], BF16, tag="wt")
                for b in range(S // 512):
                    wps = t_ps.tile([P, 512], BF16, tag="tps")
                    for j2 in range(4):
                        j = b * 4 + j2
                        nc.tensor.transpose(
                            wps[:, ts(j2, P)], e_t[:, ts(j, P)], ident_bf[:]
                        )
                    nc.scalar.copy(wt_t[:, ts(b, 512)], wps[:])

                # O = (e^T)^T @ V then row-scale by 1/l
                opsum = o_ps.tile([P, D], F32, tag="o_ps")
                for j in range(NT):
                    nc.tensor.matmul(
                        opsum[:],
                        wt_t[:, ts(j, P)],
                        v_all[:, ts(j, D)],
                        start=(j == 0),
                        stop=(j == NT - 1),
                    )
                o_t = o_p.tile([P, D], F32, tag="o")
                nc.vector.tensor_scalar_mul(o_t[:], opsum[:], linv[:, 0:1])
                nc.sync.dma_start(o_d[ts(i, P), :], o_t[:])

                # normalized fp32 weights -> DRAM
                w_t = w_p.tile([P, S], F32, tag="w")
                nc.vector.tensor_scalar_mul(w_t[:], e_t[:], linv[:, 0:1])
                nc.sync.dma_start(w_d[ts(i, P), :], w_t[:])

    nc.compile()
    return nc


_NC_CACHE = {}


def get_nc(S=S_FULL, D=D_FULL, use_f32r=True):
    key = (S, D, use_f32r)
    if key not in _NC_CACHE:
        _NC_CACHE[key] = build_kernel(S, D, use_f32r)
    return _NC_CACHE[key]


def kernel(query, key, value, temperature):
    query = np.asarray(query, dtype=np.float32)
    key = np.asarray(key, dtype=np.float32)
    value = np.asarray(value, dtype=np.float32)
    t = np.asarray(temperature, dtype=np.float32).reshape(1, 1)
    B, S, D = query.shape

    nc = get_nc(S, D)
    in_maps = [
        {
            "query": np.ascontiguousarray(query[b]),
            "key": np.ascontiguousarray(key[b]),
            "value": np.ascontiguousarray(value[b]),
            "temperature": t,
        }
        for b in range(B)
    ]
    res = run_bass_kernel_spmd(nc, in_maps, core_ids=list(range(B)))
    attended = np.stack([res.results[b]["attended"] for b in range(B)])
    weights = np.stack([res.results[b]["weights"] for b in range(B)])
    return attended, weights


# revision 18
# speedup vs baseline: 331.0942x; 1.1253x over previous
"""Euclidean-distance attention Trainium2 Bass kernel.

Problem: B=8, Sq=Sk=2048, D=512, fp32.
  dist2[q,k]  = ||Q_q||^2 + ||K_k||^2 - 2 Q_q.K_k
  W = softmax(-dist2 / temp, axis=k);  O = W @ V
Outputs: (attended [B,Sq,D], weights [B,Sq,Sk]).

Sharding: data-parallel over batch -> 1 batch element per NeuronCore (8 cores).

Per-core algorithm (softmax is invariant to the per-row constant ||Q_q||^2,
so it is dropped; mathematically identical result):
  sim[q,k] = (2 Q.K^T - ||K||^2) / temp
  W = softmax(sim, k);  O = W @ V

Engine mapping per 128-row q tile:
  PE : QK^T in float32r (1 cy/row), W^T transposes in bf16 (1 cy/row),
       W@V in bf16 (1 cy/row)
  DVE: (S_psum - ksq_bcast) subtract, row max, PV output row scale
  ACT: exp with per-partition bias = -rowmax and fused row-sum (accum_out),
       half of the PSUM->SBUF W^T copies, prolog transpose copies
  GPS: ksq broadcast, weight normalization (1/l row scale)
"""

import numpy as np
from contextlib import ExitStack

import concourse.bass as bass
import concourse.tile as tile
from concourse import bacc, mybir
from concourse.bass import ts
from concourse.bass_utils import run_bass_kernel_spmd
from concourse.masks import make_identity

F32 = mybir.dt.float32
F32R = mybir.dt.float32r
BF16 = mybir.dt.bfloat16
AX = mybir.AxisListType
AF = mybir.ActivationFunctionType

S_FULL = 2048
D_FULL = 512
N_CORES = 8


def build_kernel(S=S_FULL, D=D_FULL, use_f32r=True, repeat=1,
                 bufs_big=3, wnorm_first=False, wt_on_dve_frac=0,
                 ablate=(), nchunk=512, s_bufs=None, t_bufs=2,
                 exp_chunk=None, wt_chunk=512):
    """Build the single-core program (one batch element)."""
    P = 128
    NT = S // P          # 128-row tiles along Sq / Sk
    DC = D // P          # 128-deep contraction chunks of d
    NN = S // nchunk     # softmax chunks of the k axis
    NMM = nchunk // 512  # matmuls per chunk (N=512 each)
    if exp_chunk is None:
        exp_chunk = nchunk
    NE = S // exp_chunk  # exp chunks
    MMDT = F32R if use_f32r else F32

    nc = bacc.Bacc("TRN2", target_bir_lowering=False, debug=False)
    q_d = nc.dram_tensor("query", [S, D], F32, kind="ExternalInput").ap()
    k_d = nc.dram_tensor("key", [S, D], F32, kind="ExternalInput").ap()
    v_d = nc.dram_tensor("value", [S, D], F32, kind="ExternalInput").ap()
    t_d = nc.dram_tensor("temperature", [1, 1], F32, kind="ExternalInput").ap()
    w_d = nc.dram_tensor("weights", [S, S], F32, kind="ExternalOutput").ap()
    o_d = nc.dram_tensor("attended", [S, D], F32, kind="ExternalOutput").ap()

    with tile.TileContext(nc) as tc, ExitStack() as ctx:
        const = ctx.enter_context(tc.tile_pool(name="const", bufs=1))
        vt_p = ctx.enter_context(tc.tile_pool(name="vt", bufs=S // P))
        kt_p = ctx.enter_context(tc.tile_pool(name="kt", bufs=S // 512))
        qt_p = ctx.enter_context(tc.tile_pool(name="qt", bufs=S // P))
        ld_p = ctx.enter_context(tc.tile_pool(name="ld", bufs=3))
        sq_p = ctx.enter_context(tc.tile_pool(name="sq", bufs=2))
        st_p = ctx.enter_context(tc.tile_pool(name="st", bufs=bufs_big + 1))
        sim_p = ctx.enter_context(tc.tile_pool(name="sim", bufs=bufs_big))
        e_p = ctx.enter_context(tc.tile_pool(name="e", bufs=bufs_big))
        w_p = ctx.enter_context(tc.tile_pool(name="w", bufs=2))
        wt_p = ctx.enter_context(tc.tile_pool(name="wtt", bufs=bufs_big))
        o_p = ctx.enter_context(tc.tile_pool(name="o", bufs=2))
        dram_p = ctx.enter_context(tc.tile_pool(name="dr", bufs=1, space="DRAM"))
        if s_bufs is None:
            s_bufs = {512: 5, 1024: 3, 2048: 1}[nchunk]
        s_ps = ctx.enter_context(tc.tile_pool(name="sps", bufs=s_bufs, space="PSUM"))
        t_ps = ctx.enter_context(tc.tile_pool(name="tps", bufs=t_bufs, space="PSUM"))
        o_ps = ctx.enter_context(tc.tile_pool(name="ops", bufs=1, space="PSUM"))

        import contextlib
        loop_cm = tc.For_i(0, repeat, 1) if repeat > 1 else contextlib.nullcontext()
        with loop_cm:
            # ---- constants / scalars ----
            ident = const.tile([P, P], F32, tag="ident")
            make_identity(nc, ident[:])
            ident_bf = const.tile([P, P], BF16, tag="ident_bf")
            nc.vector.tensor_copy(ident_bf[:], ident[:])

            t_sb = const.tile([1, 1], F32, tag="t_sb")
            inv_t = const.tile([1, 1], F32, tag="inv_t")
            inv_t_b = const.tile([P, 1], F32, tag="inv_t_b")
            two_inv_t = const.tile([P, 1], F32, tag="two_inv_t")
            nc.sync.dma_start(t_sb[:], t_d[:])
            nc.vector.reciprocal(inv_t[:], t_sb[:])
            nc.gpsimd.partition_broadcast(inv_t_b[:], inv_t[0:1, 0:1])
            nc.scalar.mul(two_inv_t[:], inv_t_b[:], 2.0)

            # ---- K: load, ||K||^2, transpose into per-512-block KT tiles ----
            # kt_nb[nb] holds [c][k_local] for k in [nb*512,(nb+1)*512)
            kt_nb = [kt_p.tile([P, DC * 512], MMDT, tag="kt_nb", name=f"ktnb{b}") for b in range(S // 512)]
            ksq = const.tile([P, NT], F32, tag="ksq")
            for t in range(NT):
                kn = ld_p.tile([P, D], F32, tag="nat")
                nc.sync.dma_start(kn[:], k_d[ts(t, P), :])
                sq = sq_p.tile([P, D], F32, tag="sqs")
                nc.scalar.activation(
                    sq[:], kn[:], AF.Square, accum_out=ksq[:, t : t + 1]
                )
                ps = s_ps.tile([P, 512], F32, tag="s")
                for c in range(DC):
                    nc.tensor.transpose(ps[:, ts(c, P)], kn[:, ts(c, P)], ident[:])
                nb, tl = t // 4, t % 4
                out_ap = kt_nb[nb][:].rearrange("p (c k) -> p c k", k=512)[
                    :, :, ts(tl, P)
                ]
                nc.scalar.activation(
                    out_ap,
                    ps[:].rearrange("p (c j) -> p c j", j=P),
                    AF.Copy,
                    scale=two_inv_t[:, 0:1],
                )

            # ---- Q: load + transpose into per-q-tile QT tiles ----
            qt_i = []
            for i in range(NT):
                qn = ld_p.tile([P, D], F32, tag="nat")
                nc.sync.dma_start(qn[:], q_d[ts(i, P), :])
                ps = s_ps.tile([P, 512], F32, tag="s")
                for c in range(DC):
                    nc.tensor.transpose(ps[:, ts(c, P)], qn[:, ts(c, P)], ident[:])
                qt = qt_p.tile([P, D], MMDT, tag="qt_i", name=f"qti{i}")
                nc.scalar.copy(qt[:], ps[:])
                qt_i.append(qt)

            # ---- V: per-tile bf16 (PV matmul rhs) ----
            v_j = []
            for t in range(NT):
                vn = ld_p.tile([P, D], F32, tag="nat")
                nc.sync.dma_start(vn[:], v_d[ts(t, P), :])
                vt = vt_p.tile([P, D], BF16, tag="v_j", name=f"vj{t}")
                nc.scalar.copy(vt[:], vn[:])
                v_j.append(vt)

            # ---- ||K||^2/temp broadcast to [128, S] (full fp32) ----
            ksq_n = const.tile([P, NT], F32, tag="ksq_n")
            nc.vector.tensor_scalar_mul(ksq_n[:], ksq[:], inv_t_b[:, 0:1])
            psk = t_ps.tile([NT, P], F32, tag="tps")
            nc.tensor.transpose(psk[:], ksq_n[:], ident[:])
            sbk = const.tile([NT, P], F32, tag="ksqT")
            nc.vector.tensor_copy(sbk[:], psk[:])
            scr = dram_p.tile([1, S], F32, tag="scr")
            nc.sync.dma_start(scr[:].rearrange("a (t p) -> (a t) p", p=P), sbk[:])
            ksq_row = const.tile([1, S], F32, tag="ksq_row")
            nc.sync.dma_start(ksq_row[:], scr[:])
            ksq_bc = const.tile([P, S], F32, tag="ksq_bc")
            nc.gpsimd.partition_broadcast(ksq_bc[:], ksq_row[:])

            # ---- main loop over q tiles ----
            for i in range(NT):
                mx = st_p.tile([P, NN], F32, tag="mx")
                l4 = st_p.tile([P, NN], F32, tag="l4")
                negm = st_p.tile([P, 1], F32, tag="negm")
                lsum = st_p.tile([P, 1], F32, tag="lsum")
                linv = st_p.tile([P, 1], F32, tag="linv")

                sim_t = sim_p.tile([P, S], F32, tag="sim")
                for n in range(NN):
                    ps = s_ps.tile([P, nchunk], F32, tag="s")
                    for m in range(NMM):
                        for c in range(DC):
                            nc.tensor.matmul(
                                ps[:, ts(m, 512)],
                                qt_i[i][:, ts(c, P)],
                                kt_nb[n * NMM + m][:, ts(c, 512)],
                                start=(c == 0),
                                stop=(c == DC - 1),
                            )
                    # sim = S - ksq/temp (frees PSUM), then row max
                    if "xsubmax" in ablate:
                        if n == 0:
                            nc.gpsimd.memset(sim_t[0:1, 0:4], 0.0)
                            nc.gpsimd.memset(mx[0:1, 0:1], 0.0)
                    else:
                        nc.vector.tensor_sub(
                            sim_t[:, ts(n, nchunk)], ps[:], ksq_bc[:, ts(n, nchunk)]
                        )
                        nc.vector.reduce_max(
                            mx[:, n : n + 1], sim_t[:, ts(n, nchunk)], axis=AX.X
                        )

                nc.vector.reduce_max(negm[:], mx[:], axis=AX.X, negate=True)

                e_t = e_p.tile([P, S], BF16, tag="e")
                if "xexp" in ablate:
                    nc.gpsimd.memset(e_t[0:1, 0:4], 0.0)
                    nc.gpsimd.memset(l4[0:1, 0:1], 0.0)
                else:
                    for n in range(NE):
                        nc.scalar.activation(
                            e_t[:, ts(n, exp_chunk)],
                            sim_t[:, ts(n, exp_chunk)],
                            AF.Exp,
                            bias=negm[:, 0:1],
                            accum_out=l4[:, n : n + 1],
                        )
                if NE > 1:
                    nc.vector.reduce_sum(lsum[:], l4[:, 0:NE], axis=AX.X)
                else:
                    nc.vector.tensor_copy(lsum[:], l4[:, 0:1])
                nc.vector.reciprocal(linv[:], lsum[:])

                def emit_wnorm():
                    w_t = w_p.tile([P, S], F32, tag="w")
                    nc.vector.tensor_scalar_mul(w_t[:], e_t[:], linv[:, 0:1])
                    if "wdma" not in ablate:
                        nc.sync.dma_start(w_d[ts(i, P), :], w_t[:])

                if wnorm_first:
                    emit_wnorm()

                # W^T tiles in bf16 (transpose unnormalized exp)
                wt_t = wt_p.tile([P, S], BF16, tag="wt")
                for b in range(S // wt_chunk):
                    wps = t_ps.tile([P, wt_chunk], BF16, tag="tps")
                    for j2 in range(wt_chunk // P):
                        j = b * (wt_chunk // P) + j2
                        if "tr" in ablate and j2 > 0:
                            continue
                        nc.tensor.transpose(
                            wps[:, ts(j2, P)], e_t[:, ts(j, P)], ident_bf[:]
                        )
                    if "xwtcp" in ablate:
                        if b == 0:
                            nc.gpsimd.memset(wt_t[0:1, 0:4], 0.0)
                    else:
                        if b < wt_on_dve_frac:
                            nc.vector.tensor_copy(wt_t[:, ts(b, wt_chunk)], wps[:])
                        else:
                            nc.scalar.copy(wt_t[:, ts(b, wt_chunk)], wps[:])

                # O = (e^T)^T @ V then row-scale by 1/l
                opsum = o_ps.tile([P, D], F32, tag="o_ps")
                if "pv" not in ablate:
                    for j in range(NT):
                        nc.tensor.matmul(
                            opsum[:],
                            wt_t[:, ts(j, P)],
                            v_j[j][:],
                            start=(j == 0),
                            stop=(j == NT - 1),
                        )
                else:
                    nc.tensor.matmul(opsum[:], wt_t[:, ts(0, P)], v_all[:, ts(0, D)])
                o_t = o_p.tile([P, D], F32, tag="o")
                nc.vector.tensor_scalar_mul(o_t[:], opsum[:], linv[:, 0:1])
                nc.sync.dma_start(o_d[ts(i, P), :], o_t[:])

                if not wnorm_first:
                    emit_wnorm()

    nc.compile()
    return nc


_NC_CACHE = {}


def get_nc(S=S_FULL, D=D_FULL, use_f32r=True):
    key = (S, D, use_f32r)
    if key not in _NC_CACHE:
        _NC_CACHE[key] = build_kernel(S, D, use_f32r)
    return _NC_CACHE[key]


def kernel(query, key, value, temperature):
    query = np.asarray(query, dtype=np.float32)
    key = np.asarray(key, dtype=np.float32)
    value = np.asarray(value, dtype=np.float32)
    t = np.asarray(temperature, dtype=np.float32).reshape(1, 1)
    B, S, D = query.shape

    nc = get_nc(S, D)
    in_maps = [
        {
            "query": np.ascontiguousarray(query[b]),
            "key": np.ascontiguousarray(key[b]),
            "value": np.ascontiguousarray(value[b]),
            "temperature": t,
        }
        for b in range(B)
    ]
    res = run_bass_kernel_spmd(nc, in_maps, core_ids=list(range(B)))
    attended = np.stack([res.results[b]["attended"] for b in range(B)])
    weights = np.stack([res.results[b]["weights"] for b in range(B)])
    return attended, weights
